# revision 26
# baseline (speedup 1.0000x reference)
"""CenterLoss layer kernel for Trainium2, 8 NeuronCores, data-parallel over batch.

Reference computation (per problem):
    onehot  = one_hot(labels, C)                       # [B, C]
    xc      = onehot @ centers                         # [B, D]  (gather)
    diff    = xc - features                            # [B, D]
    delta   = onehot^T @ diff                          # [C, D]  (scatter-add)
    counts  = onehot.sum(0) + 1                        # [C, 1]
    new_c   = centers - 0.5 * delta / counts           # [C, D]
    result  = sum(diff^2, axis=1)                      # [B, 1]

Sharding: batch split 8 ways (2048 rows/core). Each core:
  - gathers its xc rows via indirect DMA from the (replicated) centers table
  - computes its result rows and partial delta/counts
  - ReduceScatter(add) combines partials; core r receives the class slice
    [128r:128r+128] and applies the EMA update for those classes only.
Host concatenates the 8 result shards and the 8 new_centers slices.
"""

import os
import sys

import numpy as np

for _p in ("/opt/trn_rl_repo", os.path.expanduser("~/.axon_site/_ro/trn_rl_repo")):
    if os.path.isdir(_p) and _p not in sys.path:
        sys.path.append(_p)

import concourse.bass as bass
import concourse.tile as tile
from concourse import bacc, mybir
from concourse.bass_utils import run_bass_kernel_spmd


def _install_ntff_hook_shim():
    """The agent image's antenv lacks axon_hooks, so trn_boot silently skips
    installing the NTFF profile hook and trace=True degrades to no-timing.
    Recreate the registry module and install the ctypes hook ourselves."""
    try:
        import types
        import antenv
        if "antenv.axon_hooks" in sys.modules:
            return
        mod = types.ModuleType("antenv.axon_hooks")
        mod._hook = None

        def set_axon_ntff_profile_hook(h):
            mod._hook = h

        def get_axon_ntff_profile_hook():
            return mod._hook

        mod.set_axon_ntff_profile_hook = set_axon_ntff_profile_hook
        mod.get_axon_ntff_profile_hook = get_axon_ntff_profile_hook
        sys.modules["antenv.axon_hooks"] = mod
        antenv.axon_hooks = mod
        from trn_agent_boot.trn_boot import _ntff_profile_via_ctypes
        so_path = "/opt/axon/libaxon_pjrt.so"
        if os.path.exists(so_path):
            mod._hook = _ntff_profile_via_ctypes(so_path)
    except Exception:
        pass


_install_ntff_hook_shim()

F32 = mybir.dt.float32
F32R = mybir.dt.float32r   # full-rate PE dtype (tf32-like), same 4-byte layout
I32 = mybir.dt.int32

N_CORES = 8
B = 16384          # full batch
C = 1024           # num classes
D = 512            # feature dim
ALPHA = 0.5
P = 128            # partitions
BS = B // N_CORES  # 2048 batch rows per core
KT = BS // P       # 16 batch tiles per core
MT = C // P        # 8 class tiles
CS = C // N_CORES  # 128 classes per core after reduce-scatter


def _build_kernel(ctx, tc, aps):
    nc = tc.nc
    feats = aps["features_s"]        # [BS, D] f32
    labels = aps["labels_s"]         # [BS]    i32
    centers = aps["centers"]         # [C, D]  f32 (gather source)
    centers_sl = aps["centers_slice"]  # [CS, D] f32 (this core's class rows)
    result_out = aps["result_s"]     # [BS, 1] f32
    newc_out = aps["new_centers_s"]  # [CS, D] f32
    rs_in = aps["rs_in"]             # [8, P+1, D] f32 internal
    rs_out = aps["rs_out"]           # [P+1, D] f32 internal Shared

    const = ctx.enter_context(tc.tile_pool(name="const", bufs=1))
    io = ctx.enter_context(tc.tile_pool(name="io", bufs=3))
    big = ctx.enter_context(tc.tile_pool(name="big", bufs=1))
    ps = ctx.enter_context(tc.tile_pool(name="ps", bufs=1, space="PSUM"))

    # ---- constants / small loads ----
    iota = const.tile([P, C], F32, tag="iota")
    nc.gpsimd.iota(iota[:], pattern=[[1, C]], base=0, channel_multiplier=0,
                   allow_small_or_imprecise_dtypes=True)
    ones_f32 = const.tile([P, 1], F32, tag="ones_f")
    nc.vector.memset(ones_f32[:], 1.0)
    ones_col = const.tile([P, 1], F32R, tag="ones")
    nc.vector.tensor_copy(out=ones_col[:], in_=ones_f32[:])
    zpad = const.tile([1, D - P], F32, tag="zpad")
    nc.vector.memset(zpad[:], 0.0)

    labels_i = const.tile([P, KT], I32, tag="labi")
    nc.sync.dma_start(out=labels_i[:], in_=labels.rearrange("(k p) -> p k", p=P))
    labels_f = const.tile([P, KT], F32, tag="labf")
    nc.vector.tensor_copy(out=labels_f[:], in_=labels_i[:])

    cent_sl = const.tile([CS, D], F32, tag="csl")
    nc.sync.dma_start(out=cent_sl[:], in_=centers_sl[:])

    # ---- gathers: xc[k] = centers[labels[k*P:(k+1)*P], :] (one offset per
    # partition per indirect DMA -- HW honors only [P, 1] offset tables) ----
    xc_tiles = []
    for k in range(KT):
        xc = big.tile([P, D], F32, tag="xc", bufs=6, name=f"xc{k}")
        nc.gpsimd.indirect_dma_start(
            out=xc[:],
            out_offset=None,
            in_=centers[:],
            in_offset=bass.IndirectOffsetOnAxis(ap=labels_i[:, k:k + 1], axis=0),
        )
        xc_tiles.append(xc)

    def xc_tile(k):
        return xc_tiles[k][:]

    # ---- features stream + onehot build ----
    feat_tiles = []
    oh_tiles = []
    for k in range(KT):
        ft = big.tile([P, D], F32, tag="ft", bufs=4, name=f"ft{k}")
        nc.sync.dma_start(out=ft[:], in_=feats[k * P:(k + 1) * P, :])
        feat_tiles.append(ft)
        oh = big.tile([P, C], F32R, tag=f"oh{k}", name=f"oh{k}")
        nc.vector.tensor_scalar(
            out=oh[:], in0=iota[:], scalar1=labels_f[:, k:k + 1], scalar2=None,
            op0=mybir.AluOpType.is_equal,
        )
        oh_tiles.append(oh)

    # ---- counts: ones^T @ onehot -> [1, C] accumulated over k ----
    ps_cnt = [ps.tile([1, D], F32, tag=f"cnt{h}", name=f"cnt{h}") for h in range(2)]
    for k in range(KT):
        for h in range(2):
            nc.tensor.matmul(
                out=ps_cnt[h][:], lhsT=ones_col[:],
                rhs=oh_tiles[k][:, h * D:(h + 1) * D],
                start=(k == 0), stop=(k == KT - 1),
            )
    counts_sb = const.tile([1, C], F32, tag="cntsb")
    for h in range(2):
        nc.vector.tensor_copy(out=counts_sb[:, h * D:(h + 1) * D], in_=ps_cnt[h][:])
    # counts slice for rank m -> rs_in[m, P, 0:P]; zero-pad the rest of the row
    for m in range(MT):
        nc.sync.dma_start(out=rs_in[m, P, 0:P],
                          in_=counts_sb[0:1, m * P:(m + 1) * P])
        nc.sync.dma_start(out=rs_in[m, P, P:D], in_=zpad[0:1, :])

    stage = int(os.environ.get("CL_STAGE", "9"))
    if stage < 2:
        return

    # ---- diff + result ----
    # diff is produced directly in f32r (PE full-rate dtype); result uses the
    # scalar engine's fused square+accumulate so the vector engine only does
    # one op per tile.
    diff_tiles = []
    for k in range(KT):
        dfr = big.tile([P, D], F32R, tag=f"dfr{k}", name=f"dfr{k}")
        nc.vector.tensor_tensor(out=dfr[:], in0=xc_tile(k), in1=feat_tiles[k][:],
                                op=mybir.AluOpType.subtract)
        diff_tiles.append(dfr)
        sq = io.tile([P, D], F32, tag="sq", name=f"sq{k}")
        res = io.tile([P, 1], F32, tag="res", name=f"res{k}")
        nc.scalar.activation(out=sq[:], in_=dfr[:],
                             func=mybir.ActivationFunctionType.Square,
                             accum_out=res[:])
        nc.sync.dma_start(out=result_out[k * P:(k + 1) * P, :], in_=res[:])

    if stage < 3:
        return

    # ---- delta: onehot^T @ diff, [C, D], in two PSUM phases of 4 class-tiles ----
    for g in range(2):
        ps_d = [ps.tile([P, D], F32, tag=f"d{m}", name=f"d{g}_{m}") for m in range(4)]
        for k in range(KT):
            for m in range(4):
                cm = g * 4 + m
                nc.tensor.matmul(
                    out=ps_d[m][:],
                    lhsT=oh_tiles[k][:, cm * P:(cm + 1) * P],
                    rhs=diff_tiles[k][:],
                    start=(k == 0), stop=(k == KT - 1),
                )
        for m in range(4):
            cm = g * 4 + m
            dsb = io.tile([P, D], F32, tag="dsb", bufs=5, name=f"dsb{cm}")
            nc.vector.tensor_copy(out=dsb[:], in_=ps_d[m][:])
            nc.sync.dma_start(out=rs_in[cm, 0:P, :], in_=dsb[:])

    if stage < 4:
        return

    # ---- cross-core reduce-scatter of [delta | counts] ----
    if os.environ.get("CL_NO_COLLECTIVE"):
        # debug mode: bypass the collective (wrong result, rank-0 chunk only)
        byp = io.tile([P, D], F32, tag="byp", name="byp0")
        nc.sync.dma_start(out=byp[:], in_=rs_in[0, 0:P, :])
        nc.sync.dma_start(out=rs_out[0:P, :], in_=byp[:])
        byp1 = io.tile([1, D], F32, tag="byp1", name="byp1")
        nc.sync.dma_start(out=byp1[:], in_=rs_in[0, P:P + 1, :])
        nc.sync.dma_start(out=rs_out[P:P + 1, :], in_=byp1[:])
    else:
        nc.gpsimd.collective_compute(
            "ReduceScatter",
            mybir.AluOpType.add,
            replica_groups=[list(range(N_CORES))],
            ins=[rs_in[:]],
            outs=[rs_out[:]],
        )

    # ---- EMA update of this core's class slice ----
    delta_r = const.tile([CS, D], F32, tag="dr")
    nc.sync.dma_start(out=delta_r[:], in_=rs_out[0:P, :])
    counts_r = const.tile([CS, 1], F32, tag="cr")
    nc.sync.dma_start(out=counts_r[:], in_=rs_out[P, 0:P][:, None])
    nc.vector.tensor_scalar_add(out=counts_r[:], in0=counts_r[:], scalar1=1.0)
    negrecip = const.tile([CS, 1], F32, tag="negrecip")
    nc.vector.reciprocal(out=negrecip[:], in_=counts_r[:])
    nc.vector.tensor_scalar_mul(out=negrecip[:], in0=negrecip[:], scalar1=-ALPHA)
    newc = const.tile([CS, D], F32, tag="newc")
    nc.vector.scalar_tensor_tensor(
        out=newc[:], in0=delta_r[:], scalar=negrecip[:], in1=cent_sl[:],
        op0=mybir.AluOpType.mult, op1=mybir.AluOpType.add,
    )
    nc.sync.dma_start(out=newc_out[:], in_=newc[:])


def _build():
    nc = bacc.Bacc("TRN2", target_bir_lowering=False, debug=False,
                   num_devices=N_CORES)
    aps = {}
    aps["features_s"] = nc.dram_tensor("features_s", [BS, D], F32,
                                       kind="ExternalInput").ap()
    aps["labels_s"] = nc.dram_tensor("labels_s", [BS], I32,
                                     kind="ExternalInput").ap()
    aps["centers"] = nc.dram_tensor("centers", [C, D], F32,
                                    kind="ExternalInput").ap()
    aps["centers_slice"] = nc.dram_tensor("centers_slice", [CS, D], F32,
                                          kind="ExternalInput").ap()
    aps["result_s"] = nc.dram_tensor("result_s", [BS, 1], F32,
                                     kind="ExternalOutput").ap()
    aps["new_centers_s"] = nc.dram_tensor("new_centers_s", [CS, D], F32,
                                          kind="ExternalOutput").ap()
    aps["rs_in"] = nc.dram_tensor("rs_in", [N_CORES, P + 1, D], F32).ap()
    aps["rs_out"] = nc.dram_tensor("rs_out", [P + 1, D], F32).ap()
    from contextlib import ExitStack
    with tile.TileContext(nc) as tc:
        with ExitStack() as ctx:
            _build_kernel(ctx, tc, aps)
    nc.compile()
    return nc


_NC_CACHE = None


def _get_nc():
    global _NC_CACHE
    if _NC_CACHE is None:
        _NC_CACHE = _build()
    return _NC_CACHE


def kernel(features=None, centers=None, labels=None, trace=False, **_unused):
    features = np.ascontiguousarray(np.asarray(features), dtype=np.float32)
    centers = np.ascontiguousarray(np.asarray(centers), dtype=np.float32)
    labels = np.ascontiguousarray(np.asarray(labels).astype(np.int32))
    assert features.shape == (B, D) and centers.shape == (C, D)
    assert labels.shape == (B,)

    nc = _get_nc()
    in_maps = []
    for r in range(N_CORES):
        in_maps.append({
            "features_s": features[r * BS:(r + 1) * BS],
            "labels_s": labels[r * BS:(r + 1) * BS],
            "centers": centers,
            "centers_slice": centers[r * CS:(r + 1) * CS],
        })
    res = run_bass_kernel_spmd(nc, in_maps, list(range(N_CORES)), trace=trace)
    result = np.concatenate([res.results[r]["result_s"] for r in range(N_CORES)],
                            axis=0)
    new_centers = np.concatenate(
        [res.results[r]["new_centers_s"] for r in range(N_CORES)], axis=0)
    if trace:
        kernel.last_exec_time_ns = res.exec_time_ns
        kernel.last_mean_exec_time_ns = res.mean_exec_time_ns
        kernel.last_results = res
    return result, new_centers


# revision 32
# speedup vs baseline: 1.1540x; 1.1540x over previous
"""CenterLoss layer kernel for Trainium2, 8 NeuronCores, data-parallel over batch.

Reference computation (per problem):
    onehot  = one_hot(labels, C)                       # [B, C]
    xc      = onehot @ centers                         # [B, D]  (gather)
    diff    = xc - features                            # [B, D]
    delta   = onehot^T @ diff                          # [C, D]  (scatter-add)
    counts  = onehot.sum(0) + 1                        # [C, 1]
    new_c   = centers - 0.5 * delta / counts           # [C, D]
    result  = sum(diff^2, axis=1)                      # [B, 1]

Sharding: batch split 8 ways (2048 rows/core). Each core:
  - gathers its xc rows via indirect DMA from the (replicated) centers table
  - computes its result rows and partial delta/counts
  - ReduceScatter(add) combines partials; core r receives the class slice
    [128r:128r+128] and applies the EMA update for those classes only.
Host concatenates the 8 result shards and the 8 new_centers slices.
"""

import os
import sys

import numpy as np

for _p in ("/opt/trn_rl_repo", os.path.expanduser("~/.axon_site/_ro/trn_rl_repo")):
    if os.path.isdir(_p) and _p not in sys.path:
        sys.path.append(_p)

import concourse.bass as bass
import concourse.tile as tile
from concourse import bacc, mybir
from concourse.bass_utils import run_bass_kernel_spmd


def _install_ntff_hook_shim():
    """The agent image's antenv lacks axon_hooks, so trn_boot silently skips
    installing the NTFF profile hook and trace=True degrades to no-timing.
    Recreate the registry module and install the ctypes hook ourselves."""
    try:
        import types
        import antenv
        if "antenv.axon_hooks" in sys.modules:
            return
        mod = types.ModuleType("antenv.axon_hooks")
        mod._hook = None

        def set_axon_ntff_profile_hook(h):
            mod._hook = h

        def get_axon_ntff_profile_hook():
            return mod._hook

        mod.set_axon_ntff_profile_hook = set_axon_ntff_profile_hook
        mod.get_axon_ntff_profile_hook = get_axon_ntff_profile_hook
        sys.modules["antenv.axon_hooks"] = mod
        antenv.axon_hooks = mod
        from trn_agent_boot.trn_boot import _ntff_profile_via_ctypes
        so_path = "/opt/axon/libaxon_pjrt.so"
        if os.path.exists(so_path):
            mod._hook = _ntff_profile_via_ctypes(so_path)
    except Exception:
        pass


_install_ntff_hook_shim()

F32 = mybir.dt.float32
F32R = mybir.dt.float32r   # full-rate PE dtype (tf32-like), same 4-byte layout
I32 = mybir.dt.int32

N_CORES = 8
B = 16384          # full batch
C = 1024           # num classes
D = 512            # feature dim
ALPHA = 0.5
P = 128            # partitions
BS = B // N_CORES  # 2048 batch rows per core
KT = BS // P       # 16 batch tiles per core
MT = C // P        # 8 class tiles
CS = C // N_CORES  # 128 classes per core after reduce-scatter


def _build_kernel(ctx, tc, aps):
    nc = tc.nc
    feats = aps["features_s"]        # [BS, D] f32
    labels = aps["labels_s"]         # [BS]    i32
    centers = aps["centers"]         # [C, D]  f32 (gather source)
    centers_sl = aps["centers_slice"]  # [CS, D] f32 (this core's class rows)
    result_out = aps["result_s"]     # [BS, 1] f32
    newc_out = aps["new_centers_s"]  # [CS, D] f32
    rs_in = aps["rs_in"]             # [8, P+1, D] f32 internal
    rs_out = aps["rs_out"]           # [P+1, D] f32 internal Shared

    const = ctx.enter_context(tc.tile_pool(name="const", bufs=1))
    io = ctx.enter_context(tc.tile_pool(name="io", bufs=3))
    big = ctx.enter_context(tc.tile_pool(name="big", bufs=1))
    ps = ctx.enter_context(tc.tile_pool(name="ps", bufs=1, space="PSUM"))

    # ---- constants / small loads ----
    iota = const.tile([P, C], F32, tag="iota")
    nc.gpsimd.iota(iota[:], pattern=[[1, C]], base=0, channel_multiplier=0,
                   allow_small_or_imprecise_dtypes=True)
    ones_f32 = const.tile([P, 1], F32, tag="ones_f")
    nc.vector.memset(ones_f32[:], 1.0)
    ones_col = const.tile([P, 1], F32R, tag="ones")
    nc.vector.tensor_copy(out=ones_col[:], in_=ones_f32[:])
    zpad = const.tile([1, D - P], F32, tag="zpad")
    nc.vector.memset(zpad[:], 0.0)

    labels_i = const.tile([P, KT], I32, tag="labi")
    nc.sync.dma_start(out=labels_i[:], in_=labels.rearrange("(k p) -> p k", p=P))
    labels_f = const.tile([P, KT], F32, tag="labf")
    nc.vector.tensor_copy(out=labels_f[:], in_=labels_i[:])

    cent_sl = const.tile([CS, D], F32, tag="csl")
    nc.sync.dma_start(out=cent_sl[:], in_=centers_sl[:])

    # ---- features stream (loaded as f32r view; PE rounds on consume) ----
    # featsum = onehot^T @ features is the only thing the reduce-scatter needs,
    # and it does NOT depend on the gather: delta = counts*centers - featsum
    # is reassembled per class slice after the collective.
    feat_tiles = []
    featr_tiles = []
    oh_tiles = []
    for k in range(KT):
        ft = big.tile([P, D], F32, tag=f"ft{k}", name=f"ft{k}")
        nc.sync.dma_start(out=ft[:], in_=feats[k * P:(k + 1) * P, :])
        feat_tiles.append(ft)
        ftr = big.tile([P, D], F32R, tag=f"ftr{k}", name=f"ftr{k}")
        nc.vector.tensor_copy(out=ftr[:], in_=ft[:])
        featr_tiles.append(ftr)
        oh = big.tile([P, C], F32R, tag=f"oh{k}", name=f"oh{k}")
        nc.vector.tensor_scalar(
            out=oh[:], in0=iota[:], scalar1=labels_f[:, k:k + 1], scalar2=None,
            op0=mybir.AluOpType.is_equal,
        )
        oh_tiles.append(oh)

    # ---- counts: ones^T @ onehot -> [1, C] accumulated over k ----
    ps_cnt = [ps.tile([1, D], F32, tag=f"cnt{h}", name=f"cnt{h}") for h in range(2)]
    for k in range(KT):
        for h in range(2):
            nc.tensor.matmul(
                out=ps_cnt[h][:], lhsT=ones_col[:],
                rhs=oh_tiles[k][:, h * D:(h + 1) * D],
                start=(k == 0), stop=(k == KT - 1),
            )
    counts_sb = const.tile([1, C], F32, tag="cntsb")
    for h in range(2):
        nc.vector.tensor_copy(out=counts_sb[:, h * D:(h + 1) * D], in_=ps_cnt[h][:])
    # counts slice for rank m -> rs_in[m, P, 0:P]; zero-pad the rest of the row
    for m in range(MT):
        nc.sync.dma_start(out=rs_in[m, P, 0:P],
                          in_=counts_sb[0:1, m * P:(m + 1) * P])
        nc.sync.dma_start(out=rs_in[m, P, P:D], in_=zpad[0:1, :])

    stage = int(os.environ.get("CL_STAGE", "9"))
    if stage < 3:
        return

    # ---- featsum: onehot^T @ features, [C, D], two PSUM phases of 4 tiles ----
    for g in range(2):
        ps_d = [ps.tile([P, D], F32, tag=f"d{m}", name=f"d{g}_{m}") for m in range(4)]
        for k in range(KT):
            for m in range(4):
                cm = g * 4 + m
                nc.tensor.matmul(
                    out=ps_d[m][:],
                    lhsT=oh_tiles[k][:, cm * P:(cm + 1) * P],
                    rhs=featr_tiles[k][:],
                    start=(k == 0), stop=(k == KT - 1),
                )
        for m in range(4):
            cm = g * 4 + m
            dsb = io.tile([P, D], F32, tag="dsb", bufs=5, name=f"dsb{cm}")
            nc.vector.tensor_copy(out=dsb[:], in_=ps_d[m][:])
            nc.sync.dma_start(out=rs_in[cm, 0:P, :], in_=dsb[:])

    if stage < 2:
        return

    # ---- gather + diff + result (parallel to the featsum/RS path) ----
    # xc[k] = centers[labels[k*P:(k+1)*P], :]; one offset per partition per
    # indirect DMA -- HW honors only [P, 1] offset tables.
    for k in range(KT):
        xc = big.tile([P, D], F32, tag="xc", bufs=6, name=f"xc{k}")
        nc.gpsimd.indirect_dma_start(
            out=xc[:],
            out_offset=None,
            in_=centers[:],
            in_offset=bass.IndirectOffsetOnAxis(ap=labels_i[:, k:k + 1], axis=0),
        )
        df = io.tile([P, D], F32, tag="df", bufs=3, name=f"df{k}")
        nc.vector.tensor_tensor(out=df[:], in0=xc[:], in1=feat_tiles[k][:],
                                op=mybir.AluOpType.subtract)
        sq = io.tile([P, D], F32, tag="sq", name=f"sq{k}")
        res = io.tile([P, 1], F32, tag="res", name=f"res{k}")
        nc.scalar.activation(out=sq[:], in_=df[:],
                             func=mybir.ActivationFunctionType.Square,
                             accum_out=res[:])
        nc.sync.dma_start(out=result_out[k * P:(k + 1) * P, :], in_=res[:])

    if stage < 4:
        return

    # ---- cross-core reduce-scatter of [delta | counts] ----
    if os.environ.get("CL_NO_COLLECTIVE"):
        # debug mode: bypass the collective (wrong result, rank-0 chunk only)
        byp = io.tile([P, D], F32, tag="byp", name="byp0")
        nc.sync.dma_start(out=byp[:], in_=rs_in[0, 0:P, :])
        nc.sync.dma_start(out=rs_out[0:P, :], in_=byp[:])
        byp1 = io.tile([1, D], F32, tag="byp1", name="byp1")
        nc.sync.dma_start(out=byp1[:], in_=rs_in[0, P:P + 1, :])
        nc.sync.dma_start(out=rs_out[P:P + 1, :], in_=byp1[:])
    else:
        nc.gpsimd.collective_compute(
            "ReduceScatter",
            mybir.AluOpType.add,
            replica_groups=[list(range(N_CORES))],
            ins=[rs_in[:]],
            outs=[rs_out[:]],
        )

    # ---- EMA update of this core's class slice ----
    # delta = counts*centers - featsum, so
    # new = c - ALPHA*(counts*c - fs)/(counts+1) = (1 - ALPHA*w)*c + ALPHA*r*fs
    # with r = 1/(counts+1), w = counts*r.
    fs_r = const.tile([CS, D], F32, tag="fsr")
    nc.sync.dma_start(out=fs_r[:], in_=rs_out[0:P, :])
    counts_r = const.tile([CS, 1], F32, tag="cr")
    nc.sync.dma_start(out=counts_r[:], in_=rs_out[P, 0:P][:, None])
    counts_p1 = const.tile([CS, 1], F32, tag="cp1")
    nc.vector.tensor_scalar_add(out=counts_p1[:], in0=counts_r[:], scalar1=1.0)
    rec = const.tile([CS, 1], F32, tag="rec")
    nc.vector.reciprocal(out=rec[:], in_=counts_p1[:])
    w = const.tile([CS, 1], F32, tag="w")
    nc.vector.tensor_tensor(out=w[:], in0=counts_r[:], in1=rec[:],
                            op=mybir.AluOpType.mult)
    a1 = const.tile([CS, 1], F32, tag="a1")
    nc.vector.tensor_scalar(out=a1[:], in0=w[:], scalar1=-ALPHA, scalar2=1.0,
                            op0=mybir.AluOpType.mult, op1=mybir.AluOpType.add)
    ar = const.tile([CS, 1], F32, tag="ar")
    nc.vector.tensor_scalar_mul(out=ar[:], in0=rec[:], scalar1=ALPHA)
    tmp = const.tile([CS, D], F32, tag="tmp")
    nc.vector.tensor_scalar(out=tmp[:], in0=cent_sl[:], scalar1=a1[:, 0:1],
                            scalar2=None, op0=mybir.AluOpType.mult)
    newc = const.tile([CS, D], F32, tag="newc")
    nc.vector.scalar_tensor_tensor(
        out=newc[:], in0=fs_r[:], scalar=ar[:, 0:1], in1=tmp[:],
        op0=mybir.AluOpType.mult, op1=mybir.AluOpType.add,
    )
    nc.sync.dma_start(out=newc_out[:], in_=newc[:])


def _build():
    nc = bacc.Bacc("TRN2", target_bir_lowering=False, debug=False,
                   num_devices=N_CORES)
    aps = {}
    aps["features_s"] = nc.dram_tensor("features_s", [BS, D], F32,
                                       kind="ExternalInput").ap()
    aps["labels_s"] = nc.dram_tensor("labels_s", [BS], I32,
                                     kind="ExternalInput").ap()
    aps["centers"] = nc.dram_tensor("centers", [C, D], F32,
                                    kind="ExternalInput").ap()
    aps["centers_slice"] = nc.dram_tensor("centers_slice", [CS, D], F32,
                                          kind="ExternalInput").ap()
    aps["result_s"] = nc.dram_tensor("result_s", [BS, 1], F32,
                                     kind="ExternalOutput").ap()
    aps["new_centers_s"] = nc.dram_tensor("new_centers_s", [CS, D], F32,
                                          kind="ExternalOutput").ap()
    aps["rs_in"] = nc.dram_tensor("rs_in", [N_CORES, P + 1, D], F32).ap()
    aps["rs_out"] = nc.dram_tensor("rs_out", [P + 1, D], F32).ap()
    from contextlib import ExitStack
    with tile.TileContext(nc) as tc:
        with ExitStack() as ctx:
            _build_kernel(ctx, tc, aps)
    nc.compile()
    return nc


_NC_CACHE = None


def _get_nc():
    global _NC_CACHE
    if _NC_CACHE is None:
        _NC_CACHE = _build()
    return _NC_CACHE


def kernel(features=None, centers=None, labels=None, trace=False, **_unused):
    features = np.ascontiguousarray(np.asarray(features), dtype=np.float32)
    centers = np.ascontiguousarray(np.asarray(centers), dtype=np.float32)
    labels = np.ascontiguousarray(np.asarray(labels).astype(np.int32))
    assert features.shape == (B, D) and centers.shape == (C, D)
    assert labels.shape == (B,)

    nc = _get_nc()
    in_maps = []
    for r in range(N_CORES):
        in_maps.append({
            "features_s": features[r * BS:(r + 1) * BS],
            "labels_s": labels[r * BS:(r + 1) * BS],
            "centers": centers,
            "centers_slice": centers[r * CS:(r + 1) * CS],
        })
    res = run_bass_kernel_spmd(nc, in_maps, list(range(N_CORES)), trace=trace)
    result = np.concatenate([res.results[r]["result_s"] for r in range(N_CORES)],
                            axis=0)
    new_centers = np.concatenate(
        [res.results[r]["new_centers_s"] for r in range(N_CORES)], axis=0)
    if trace:
        kernel.last_exec_time_ns = res.exec_time_ns
        kernel.last_mean_exec_time_ns = res.mean_exec_time_ns
        kernel.last_results = res
    return result, new_centers
